# revision 5
# baseline (speedup 1.0000x reference)
"""GNN L1 aggregator: safe baseline — one [128,1] indirect gather per (tile, neighbor slot)."""
import os
import sys

for _p in ("/opt/trn_rl_repo", "/opt/pypackages"):
    if _p not in sys.path and os.path.isdir(_p):
        sys.path.append(_p)

import numpy as np

NUM_AUTHOR = 131072
D = 128
N_NODES = 32768
G = 32
NCORES = 8
NPC = N_NODES // NCORES   # 4096
P = 128
TILES = NPC // P          # 32 tiles of 128 nodes
ZERO_ROW = NUM_AUTHOR

_CACHE = {}
LAST_RESULT = None


def _build_program():
    from concourse import bacc, bass, mybir
    import concourse.tile as tile

    nc = bacc.Bacc("TRN2", target_bir_lowering=False, debug=False,
                   enable_asserts=False, num_devices=NCORES)
    dt = mybir.dt
    a2e = nc.dram_tensor("a2e", [NUM_AUTHOR + 1, D], dt.float32, kind="ExternalInput")
    idx = nc.dram_tensor("idx", [P, TILES * G], dt.int32, kind="ExternalInput")
    scl = nc.dram_tensor("scl", [P, TILES], dt.float32, kind="ExternalInput")
    out = nc.dram_tensor("out", [NPC, D], dt.float32, kind="ExternalOutput")

    with tile.TileContext(nc) as tc:
        with (
            tc.tile_pool(name="persist", bufs=1) as pp,
            tc.tile_pool(name="work", bufs=2) as wp,
        ):
            idx_sb = pp.tile([P, TILES * G], dt.int32)
            scl_sb = pp.tile([P, TILES], dt.float32)
            nc.sync.dma_start(out=idx_sb[:], in_=idx[:])
            nc.sync.dma_start(out=scl_sb[:], in_=scl[:])

            for t in range(TILES):
                g = wp.tile([P, G * D], dt.float32, tag="gather")
                for j in range(G):
                    nc.gpsimd.indirect_dma_start(
                        out=g[:, j * D:(j + 1) * D],
                        out_offset=None,
                        in_=a2e[:],
                        in_offset=bass.IndirectOffsetOnAxis(
                            ap=idx_sb[:, t * G + j:t * G + j + 1],
                            axis=0,
                        ),
                    )
                r = wp.tile([P, D], dt.float32, tag="res")
                gv = g[:].rearrange("p (g d) -> p d g", g=G, d=D)
                nc.vector.tensor_reduce(
                    out=r[:], in_=gv,
                    axis=mybir.AxisListType.X, op=mybir.AluOpType.add,
                )
                sv = scl_sb[:, t:t + 1].broadcast_to([P, D])
                nc.vector.tensor_tensor(
                    out=r[:], in0=r[:], in1=sv, op=mybir.AluOpType.mult,
                )
                nc.sync.dma_start(out=out[t * P:(t + 1) * P, :], in_=r[:])

    nc.compile()
    return nc


def _prep_inputs(neighbors, lengths, a2e):
    neighbors = np.asarray(neighbors)
    lengths = np.asarray(lengths)
    a2e = np.asarray(a2e, dtype=np.float32)

    mask = np.arange(G)[None, :] < lengths[:, None]
    idx_clean = np.where(mask, neighbors, ZERO_ROW).astype(np.int32)
    inv_len = np.where(lengths > 0, 1.0 / np.maximum(lengths, 1), 0.0).astype(np.float32)

    # node(core, t, p) = core*NPC + t*P + p ; column layout (t, g)
    idx_dram = (
        idx_clean.reshape(NCORES, TILES, P, G)
        .transpose(0, 2, 1, 3)
        .reshape(NCORES, P, TILES * G)
    )
    scl_dram = (
        inv_len.reshape(NCORES, TILES, P)
        .transpose(0, 2, 1)
        .reshape(NCORES, P, TILES)
    )
    a2e_pad = np.concatenate([a2e, np.zeros((1, D), np.float32)], axis=0)
    return idx_dram, scl_dram, a2e_pad


def _install_ntff_hook_shim():
    import types
    if "antenv.axon_hooks" in sys.modules:
        return
    from trn_agent_boot.trn_boot import _ntff_profile_via_ctypes
    hook = _ntff_profile_via_ctypes("/opt/axon/libaxon_pjrt.so")
    mod = types.ModuleType("antenv.axon_hooks")
    mod._hook = hook
    mod.get_axon_ntff_profile_hook = lambda: mod._hook
    mod.set_axon_ntff_profile_hook = lambda h: setattr(mod, "_hook", h)
    sys.modules["antenv.axon_hooks"] = mod


def kernel(node, neighbors, lengths, a2e, _trace=False):
    global LAST_RESULT
    from concourse.bass_utils import run_bass_kernel_spmd

    if _trace:
        try:
            _install_ntff_hook_shim()
            import concourse.bass_utils as _bu
            _bu.upload_artifacts = lambda tmpdir: f"local://{tmpdir}"
        except Exception as e:
            print(f"ntff hook shim failed ({e}); running without trace")
            _trace = False

    if "nc" not in _CACHE:
        _CACHE["nc"] = _build_program()
    nc = _CACHE["nc"]

    idx_dram, scl_dram, a2e_pad = _prep_inputs(neighbors, lengths, a2e)
    in_maps = [
        {
            "a2e": np.ascontiguousarray(a2e_pad),
            "idx": np.ascontiguousarray(idx_dram[c]),
            "scl": np.ascontiguousarray(scl_dram[c]),
        }
        for c in range(NCORES)
    ]
    res = run_bass_kernel_spmd(nc, in_maps, list(range(NCORES)), trace=_trace)
    LAST_RESULT = res
    out = np.concatenate([res.results[c]["out"] for c in range(NCORES)], axis=0)
    return out
